# revision 1
# baseline (speedup 1.0000x reference)
"""Multi-head causal self-attention on 8 TRN2 NeuronCores.

Sharding: data parallel over batch (2) x tensor parallel over heads (16 -> 4
groups of 4 heads).  Core c handles batch c//4 and heads 4*(c%4) .. 4*(c%4)+3.
Each core computes a partial output-projection (its 4 heads' contribution,
[S, D]); the host sums the 4 partials per batch and adds the biases.
No device collectives needed.

Per-core device program (all matmul compute in bf16, f32 PSUM accumulate):
  P1: QT,KT = (x @ WqT, x @ WkT) produced transposed [e, s] (e = head*128+dk
      on partitions); V produced natural [s, e] with a ones column appended
      per head (gives softmax denominators for free in the AV matmul).
  P2/P3 fused, i-block-major: for each 512-wide i-block, per head-pair:
      scoresT[j,i] for both heads in one [128,1024] PSUM tile (single K=128
      matmuls, causal blocks only), one exp on ACT per tile (no max
      subtraction: scores are O(1) by construction), AV matmuls with expT
      slices directly as the stationary operand (softmax denominator comes
      from V's ones column), per-partition reciprocal + scale; then the
      4 finished s-tiles of O are PE-transposed and projected through WoT
      immediately, interleaving with the next i-block's attention.

Host folds: 1/sqrt(dk) into Wq/bq; V-bias contribution = wo @ bv (rows of a
softmax sum to exactly 1) and bo are added on the host.  Output partials are
bf16 (summed in f32 on the host).
"""

import numpy as np
import ml_dtypes
from contextlib import ExitStack

import concourse.bass as bass
import concourse.mybir as mybir
import concourse.tile as tile
from concourse import bacc
from concourse.bass_utils import run_bass_kernel_spmd
from concourse.masks import make_upper_triangular, make_identity

BF16 = ml_dtypes.bfloat16
F32 = mybir.dt.float32
BF = mybir.dt.bfloat16

B = 2
S = 2048
D = 2048
H = 16
DK = 128
NCORES = 8
HPC = 4                  # heads per core
E = HPC * DK             # 512 = output cols per core for q/k/v
P = 128
NDC = D // P             # 16 d-chunks
NST = S // P             # 16 s-tiles
NSB = S // 512           # 4 s/i blocks of 512
DKP = DK + 1             # dk + ones column
N_WARMUP = 44            # dummy matmuls to warm the PE HAM during DMA ramp


def _build_nc():
    nc = bacc.Bacc("TRN2", target_bir_lowering=False, debug=False)

    xt = nc.dram_tensor("xt", [D, S], BF, kind="ExternalInput").ap()
    wqt = nc.dram_tensor("wqt", [D, E], BF, kind="ExternalInput").ap()
    wkt = nc.dram_tensor("wkt", [D, E], BF, kind="ExternalInput").ap()
    wvt = nc.dram_tensor("wvt", [D, E], BF, kind="ExternalInput").ap()
    wot = nc.dram_tensor("wot", [E, D], BF, kind="ExternalInput").ap()
    bqd = nc.dram_tensor("bq", [E], F32, kind="ExternalInput").ap()
    bkd = nc.dram_tensor("bk", [E], F32, kind="ExternalInput").ap()
    outd = nc.dram_tensor("out", [S, D], BF, kind="ExternalOutput").ap()

    with tile.TileContext(nc) as tc, ExitStack() as ctx:
        # PSUM: sc 2x[128,1024] (4 banks) + acc 2x[128,512] (2) + tp 2 = 8
        pst = ctx.enter_context(tc.tile_pool(name="pst", bufs=2, space="PSUM"))
        persist = ctx.enter_context(tc.tile_pool(name="persist", bufs=1))

        qt_t = [persist.tile([P, S], BF, name=f"qt{h}", tag=f"qt{h}") for h in range(HPC)]
        kt_t = [persist.tile([P, S], BF, name=f"kt{h}", tag=f"kt{h}") for h in range(HPC)]
        v_t = [persist.tile([P, HPC, DKP], BF, name=f"v{j}", tag=f"v{j}") for j in range(NST)]
        tri = persist.tile([P, P], BF, name="tri", tag="tri")
        ident = persist.tile([P, P], BF, name="ident", tag="ident")
        bq_sb = persist.tile([P, HPC], F32, name="bq_sb", tag="bq_sb")
        bk_sb = persist.tile([P, HPC], F32, name="bk_sb", tag="bk_sb")

        # PE warmup during the input-DMA ramp (results are never read); the
        # operand is produced by a single fast DVE memset, not gpsimd.
        wupd = persist.tile([P, P], BF, name="wupd", tag="wupd")
        nc.vector.memset(wupd[:], 0.0)
        for i in range(N_WARMUP):
            pw = pst.tile([P, 512], F32, name="pw", tag="acc", bufs=4)
            nc.tensor.matmul(pw[:, 0:P], wupd[:], wupd[:], start=True, stop=True)
        # preload the ACT Exp function table now, off the first-score path
        dexp = persist.tile([P, 1], F32, name="dexp", tag="dexp")
        nc.scalar.activation(dexp[:], wupd[:, 0:1],
                             mybir.ActivationFunctionType.Exp)

        # tri[p, f] = 1.0 iff p <= f  (keep j <= i on the diagonal block)
        make_upper_triangular(nc, tri[:], val=1.0, diag=True)
        make_identity(nc, ident[:])
        nc.sync.dma_start(bq_sb[:], bqd.rearrange("(o p) -> p o", p=P))
        nc.sync.dma_start(bk_sb[:], bkd.rearrange("(o p) -> p o", p=P))
        for j in range(NST):
            nc.vector.memset(v_t[j][:, :, DK:DKP], 1.0)

        # ------------------------------------------------------------------
        # Phase 1: QT/KT [e, s] and V [s, e]
        # ------------------------------------------------------------------
        with tc.tile_pool(name="p1", bufs=1) as p1:
            xt_t = [p1.tile([P, S], BF, name=f"xt{dc}", tag=f"xt{dc}") for dc in range(NDC)]
            wq_t = [p1.tile([P, E], BF, name=f"wq{dc}", tag=f"wq{dc}") for dc in range(NDC)]
            wk_t = [p1.tile([P, E], BF, name=f"wk{dc}", tag=f"wk{dc}") for dc in range(NDC)]
            wv_t = [p1.tile([P, E], BF, name=f"wv{dc}", tag=f"wv{dc}") for dc in range(NDC)]
            # wq + xt first (gate the Q matmuls); wk/wv are needed later.
            for dc in range(NDC):
                sl = slice(dc * P, (dc + 1) * P)
                nc.sync.dma_start(wq_t[dc][:], wqt[sl, :])
                nc.sync.dma_start(xt_t[dc][:], xt[sl, :])
            for dc in range(NDC):
                sl = slice(dc * P, (dc + 1) * P)
                nc.sync.dma_start(wk_t[dc][:], wkt[sl, :])
            for dc in range(NDC):
                sl = slice(dc * P, (dc + 1) * P)
                nc.sync.dma_start(wv_t[dc][:], wvt[sl, :])

            def qk_evac(dest, et, psA, psB, bias_sb):
                nc.vector.tensor_scalar_add(
                    dest[et][:, 0:1024], psA[:], bias_sb[:, et:et + 1])
                for i in range(2):
                    nc.vector.tensor_scalar_add(
                        dest[et][:, 1024 + i * 512:1024 + (i + 1) * 512],
                        psB[i][:], bias_sb[:, et:et + 1])

            def qk_psums(et):
                psA = pst.tile([P, 1024], F32, name=f"psA{et}", tag="sc", bufs=2)
                psB = [pst.tile([P, 512], F32, name=f"psB{et}_{i}", tag="acc",
                                bufs=4) for i in range(2)]
                outs = [psA[:, 0:512], psA[:, 512:1024], psB[0][:], psB[1][:]]
                return psA, psB, outs

            # Q: e-tile PAIRS concurrently — during the x-DMA ramp each
            # arriving chunk feeds 8 matmuls instead of 4, keeping PE busy
            for pair in range(2):
                ets = (2 * pair, 2 * pair + 1)
                ps = {et: qk_psums(et) for et in ets}
                for dc in range(NDC):
                    for et in ets:
                        lhsT = wq_t[dc][:, et * P:(et + 1) * P]
                        for sb_ in range(NSB):
                            nc.tensor.matmul(
                                ps[et][2][sb_], lhsT,
                                xt_t[dc][:, sb_ * 512:(sb_ + 1) * 512],
                                start=(dc == 0), stop=(dc == NDC - 1))
                for et in ets:
                    qk_evac(qt_t, et, ps[et][0], ps[et][1], bq_sb)

            # K: sequential e-tile groups (input already resident)
            for et in range(HPC):
                psA, psB, outs = qk_psums(et)
                for dc in range(NDC):
                    lhsT = wk_t[dc][:, et * P:(et + 1) * P]
                    for sb_ in range(NSB):
                        nc.tensor.matmul(
                            outs[sb_], lhsT,
                            xt_t[dc][:, sb_ * 512:(sb_ + 1) * 512],
                            start=(dc == 0), stop=(dc == NDC - 1))
                qk_evac(kt_t, et, psA, psB, bk_sb)

            # V: out[s_tile(128), e(512)] accumulated over d-chunks
            for st in range(NST):
                psv = pst.tile([P, 512], F32, name="psv", tag="acc", bufs=4)
                for dc in range(NDC):
                    nc.tensor.matmul(
                        psv[:], xt_t[dc][:, st * P:(st + 1) * P], wv_t[dc][:],
                        start=(dc == 0), stop=(dc == NDC - 1))
                nc.vector.tensor_copy(
                    v_t[st][:, :, 0:DK],
                    psv[:].rearrange("p (h w) -> p h w", h=HPC))

        # ------------------------------------------------------------------
        # Phase 2+3 fused, i-block-major
        # ------------------------------------------------------------------
        with tc.tile_pool(name="p2", bufs=1) as p2, \
                tc.tile_pool(name="exps", bufs=18) as epool, \
                tc.tile_pool(name="small", bufs=8) as spool, \
                tc.tile_pool(name="yout", bufs=4) as ypool:
            o_t = [p2.tile([P, HPC * DK], BF, name=f"o{st}", tag=f"o{st}")
                   for st in range(NST)]
            wot_t = [p2.tile([P, D], BF, name=f"wot{ec}", tag=f"wot{ec}")
                     for ec in range(HPC)]
            ot_t = [p2.tile([P, S], BF, name=f"ot{ec}", tag=f"ot{ec}")
                    for ec in range(HPC)]
            for ec in range(HPC):
                nc.sync.dma_start(wot_t[ec][:], wot[ec * P:(ec + 1) * P, :])

            for ib in range(NSB):
                njt = 4 * ib + 4
                for hp in range(2):          # head pair: heads (2hp, 2hp+1)
                    # scoresT + exp for this head pair: [128,1024] tiles
                    etiles = []
                    for jt in range(njt):
                        pss = pst.tile([P, 1024], F32, name="pss", tag="sc", bufs=2)
                        # band tiles only need i >= jt*128: slice N accordingly
                        c0 = max(0, (jt - 4 * ib)) * P
                        for k in range(2):
                            h = 2 * hp + k
                            nc.tensor.matmul(
                                pss[:, k * 512 + c0:(k + 1) * 512],
                                kt_t[h][:, jt * P:(jt + 1) * P],
                                qt_t[h][:, ib * 512 + c0:(ib + 1) * 512],
                                start=True, stop=True)
                        et_t = epool.tile([P, 1024], BF, name="et", tag="exp")
                        if jt < 4 * ib:
                            nc.scalar.activation(
                                et_t[:], pss[:], mybir.ActivationFunctionType.Exp)
                        else:
                            s_off = jt - 4 * ib
                            # one call over both heads' unmasked regions
                            nc.scalar.activation(
                                et_t[:, s_off * P:1024], pss[:, s_off * P:1024],
                                mybir.ActivationFunctionType.Exp)
                            # zero the diag-masked part of both heads at once
                            et3 = et_t[:].rearrange("p (h w) -> p h w", h=2)
                            nc.vector.tensor_tensor(
                                et3[:, :, s_off * P:(s_off + 1) * P],
                                et3[:, :, s_off * P:(s_off + 1) * P],
                                tri[:, None, :].to_broadcast([P, 2, P]),
                                mybir.AluOpType.mult)
                        etiles.append(et_t)
                    def av_one(h, k, t):
                        it = 4 * ib + t
                        po = pst.tile([P, 512], F32, name="po", tag="acc", bufs=4)
                        for jt in range(it + 1):
                            nc.tensor.matmul(
                                po[:, 0:DKP],
                                etiles[jt][:, k * 512 + t * P:k * 512 + (t + 1) * P],
                                v_t[jt][:, h, :],
                                start=(jt == 0), stop=(jt == it))
                        rec = spool.tile([P, 1], F32, name="rec", tag="rec")
                        nc.vector.reciprocal(rec[:], po[:, DK:DKP])
                        nc.vector.tensor_scalar_mul(
                            o_t[it][:, h * P:(h + 1) * P], po[:, 0:DK], rec[:])

                    def tp_p3_one(st):
                        for ec in range(HPC):
                            pt = pst.tile([P, P], BF, name="pt", tag="acc", bufs=4)
                            nc.tensor.transpose(
                                pt[:], o_t[st][:, ec * P:(ec + 1) * P], ident[:])
                            nc.vector.tensor_copy(
                                ot_t[ec][:, st * P:(st + 1) * P], pt[:])
                        for ob in range(NSB):
                            py = pst.tile([P, 512], F32, name="py", tag="acc", bufs=4)
                            for ec in range(HPC):
                                nc.tensor.matmul(
                                    py[:], ot_t[ec][:, st * P:(st + 1) * P],
                                    wot_t[ec][:, ob * 512:(ob + 1) * 512],
                                    start=(ec == 0), stop=(ec == HPC - 1))
                            y = ypool.tile([P, 512], BF, name="y", tag="y")
                            nc.vector.tensor_copy(y[:], py[:])
                            nc.sync.dma_start(
                                outd[st * P:(st + 1) * P,
                                     ob * 512:(ob + 1) * 512],
                                y[:])

                    if hp == 0:
                        for k in range(2):
                            for t in range(4):
                                av_one(2 * hp + k, k, t)
                    else:
                        # t-major: each finished s-tile's transpose +
                        # projection starts immediately
                        for t in range(4):
                            for k in range(2):
                                av_one(2 * hp + k, k, t)
                            tp_p3_one(4 * ib + t)

    nc.finalize()
    return nc


_NC_CACHE = {}


def _get_nc():
    if "nc" not in _NC_CACHE:
        _NC_CACHE["nc"] = _build_nc()
    return _NC_CACHE["nc"]


def _make_in_maps(x, wq, bq, wk, bk, wv, wo):
    scale = np.float32(1.0 / np.sqrt(DK))
    in_maps = []
    for c in range(NCORES):
        b = c // 4
        g = c % 4
        sl = slice(E * g, E * (g + 1))
        in_maps.append({
            "xt": np.ascontiguousarray(x[b].T).astype(BF16),
            "wqt": np.ascontiguousarray((wq[sl] * scale).T).astype(BF16),
            "wkt": np.ascontiguousarray(wk[sl].T).astype(BF16),
            "wvt": np.ascontiguousarray(wv[sl].T).astype(BF16),
            "wot": np.ascontiguousarray(wo[:, sl].T).astype(BF16),
            "bq": (bq[sl] * scale).astype(np.float32),
            "bk": bk[sl].astype(np.float32),
        })
    return in_maps


def _assemble(core_outs, wv_bias_vec):
    out = np.empty((B, S, D), np.float32)
    for b in range(B):
        acc = core_outs[4 * b].astype(np.float32)
        for g in range(1, 4):
            acc = acc + core_outs[4 * b + g].astype(np.float32)
        out[b] = acc + wv_bias_vec
    return out


def kernel(x, wq, bq, wk, bk, wv, bv, wo, bo, mask, _trace=False):
    x = np.asarray(x, dtype=np.float32)
    wq = np.asarray(wq, dtype=np.float32)
    bq = np.asarray(bq, dtype=np.float32)
    wk = np.asarray(wk, dtype=np.float32)
    bk = np.asarray(bk, dtype=np.float32)
    wv = np.asarray(wv, dtype=np.float32)
    bv = np.asarray(bv, dtype=np.float32)
    wo = np.asarray(wo, dtype=np.float32)
    bo = np.asarray(bo, dtype=np.float32)

    in_maps = _make_in_maps(x, wq, bq, wk, bk, wv, wo)
    nc = _get_nc()
    res = run_bass_kernel_spmd(nc, in_maps, core_ids=list(range(NCORES)),
                               trace=_trace)
    core_outs = [res.results[c]["out"] for c in range(NCORES)]
    # rows of softmax sum to 1 -> per-head V-bias contributes wo[:, sl] @ bv[sl]
    bias_vec = (bo + wo @ bv).astype(np.float32)
    out = _assemble(core_outs, bias_vec)
    if _trace:
        return out, res
    return out



# revision 2
# speedup vs baseline: 1.0431x; 1.0431x over previous
"""Multi-head causal self-attention on 8 TRN2 NeuronCores.

Sharding: data parallel over batch (2) x tensor parallel over heads (16 -> 4
groups of 4 heads).  Core c handles batch c//4 and heads 4*(c%4) .. 4*(c%4)+3.
Each core computes a partial output-projection (its 4 heads' contribution,
[S, D]); the host sums the 4 partials per batch and adds the biases.
No device collectives needed.

Per-core device program (all matmul compute in bf16, f32 PSUM accumulate):
  P1: QT,KT = (x @ WqT, x @ WkT) produced transposed [e, s] (e = head*128+dk
      on partitions); V produced natural [s, e] with a ones column appended
      per head (gives softmax denominators for free in the AV matmul).
  P2/P3 fused and software-pipelined ("zipper"): the scores+exp pipeline for
      section s=(i-block, head-pair) is rate-limited by the ACT engine (exp of
      a [128,1024] tile takes ~2.4x longer than its two score matmuls), so the
      PE work of section s-1 (AV matmuls, PE transposes of finished O s-tiles,
      and the output projection) is interleaved BETWEEN the score-tile matmuls
      of section s.  The PE never idles waiting for exp; exp tiles of section
      s-1 are consumed while section s's are produced (epool sized for ~2
      sections of liveness).

Host folds: 1/sqrt(dk) into Wq/bq; V-bias contribution = wo @ bv (rows of a
softmax sum to exactly 1) and bo are added on the host.  Output partials are
bf16 (summed in f32 on the host).
"""

import numpy as np
import ml_dtypes
from contextlib import ExitStack

import concourse.bass as bass
import concourse.mybir as mybir
import concourse.tile as tile
from concourse import bacc
from concourse.bass_utils import run_bass_kernel_spmd
from concourse.masks import make_upper_triangular, make_identity

BF16 = ml_dtypes.bfloat16
F32 = mybir.dt.float32
BF = mybir.dt.bfloat16

B = 2
S = 2048
D = 2048
H = 16
DK = 128
NCORES = 8
HPC = 4                  # heads per core
E = HPC * DK             # 512 = output cols per core for q/k/v
P = 128
NDC = D // P             # 16 d-chunks
NST = S // P             # 16 s-tiles
NSB = S // 512           # 4 s/i blocks of 512
DKP = DK + 1             # dk + ones column
N_WARMUP = 16            # dummy matmuls to ramp the PE p-state during DMA


def _build_nc():
    nc = bacc.Bacc("TRN2", target_bir_lowering=False, debug=False)

    xt = nc.dram_tensor("xt", [D, S], BF, kind="ExternalInput").ap()
    wqt = nc.dram_tensor("wqt", [D, E], BF, kind="ExternalInput").ap()
    wkt = nc.dram_tensor("wkt", [D, E], BF, kind="ExternalInput").ap()
    wvt = nc.dram_tensor("wvt", [D, E], BF, kind="ExternalInput").ap()
    wot = nc.dram_tensor("wot", [E, D], BF, kind="ExternalInput").ap()
    bqd = nc.dram_tensor("bq", [E], F32, kind="ExternalInput").ap()
    bkd = nc.dram_tensor("bk", [E], F32, kind="ExternalInput").ap()
    outd = nc.dram_tensor("out", [S, D], BF, kind="ExternalOutput").ap()

    with tile.TileContext(nc) as tc, ExitStack() as ctx:
        # PSUM: sc 2x[128,1024] (4 banks) + acc 4x[128,512] (4) = 8
        pst = ctx.enter_context(tc.tile_pool(name="pst", bufs=2, space="PSUM"))
        persist = ctx.enter_context(tc.tile_pool(name="persist", bufs=1))

        qt_t = [persist.tile([P, S], BF, name=f"qt{h}", tag=f"qt{h}") for h in range(HPC)]
        kt_t = [persist.tile([P, S], BF, name=f"kt{h}", tag=f"kt{h}") for h in range(HPC)]
        v_t = [persist.tile([P, HPC, DKP], BF, name=f"v{j}", tag=f"v{j}") for j in range(NST)]
        tri = persist.tile([P, P], BF, name="tri", tag="tri")
        ident = persist.tile([P, P], BF, name="ident", tag="ident")
        bq_sb = persist.tile([P, HPC], F32, name="bq_sb", tag="bq_sb")
        bk_sb = persist.tile([P, HPC], F32, name="bk_sb", tag="bk_sb")

        # small-bias DMAs first (tiny, take queues 0/1)
        nc.sync.dma_start(bq_sb[:], bqd.rearrange("(o p) -> p o", p=P))
        nc.sync.dma_start(bk_sb[:], bkd.rearrange("(o p) -> p o", p=P))

        # PE warmup during the input-DMA ramp (results are never read); the
        # operand is produced by a single fast DVE memset, not gpsimd.
        wupd = persist.tile([P, P], BF, name="wupd", tag="wupd")
        nc.vector.memset(wupd[:], 0.0)
        for i in range(N_WARMUP):
            pw = pst.tile([P, 512], F32, name="pw", tag="acc", bufs=4)
            nc.tensor.matmul(pw[:, 0:P], wupd[:], wupd[:], start=True, stop=True)
        # preload the ACT Exp function table now, off the first-score path
        dexp = persist.tile([P, 1], F32, name="dexp", tag="dexp")
        nc.scalar.activation(dexp[:], wupd[:, 0:1],
                             mybir.ActivationFunctionType.Exp)

        # tri[p, f] = 1.0 iff p <= f  (keep j <= i on the diagonal block)
        make_upper_triangular(nc, tri[:], val=1.0, diag=True)
        make_identity(nc, ident[:])
        for j in range(NST):
            nc.vector.memset(v_t[j][:, :, DK:DKP], 1.0)

        # ------------------------------------------------------------------
        # Phase 1: QT/KT [e, s] and V [s, e]
        # ------------------------------------------------------------------
        with tc.tile_pool(name="p1", bufs=1) as p1:
            xt_t = [p1.tile([P, S], BF, name=f"xt{dc}", tag=f"xt{dc}") for dc in range(NDC)]
            wq_t = [p1.tile([P, E], BF, name=f"wq{dc}", tag=f"wq{dc}") for dc in range(NDC)]
            wk_t = [p1.tile([P, E], BF, name=f"wk{dc}", tag=f"wk{dc}") for dc in range(NDC)]
            wv_t = [p1.tile([P, E], BF, name=f"wv{dc}", tag=f"wv{dc}") for dc in range(NDC)]
            # wq + xt first (gate the Q matmuls); wk/wv are needed later.
            # The first two d-chunks' x tiles are split across 4 queues each
            # so the very first Q matmul is not gated on one slow queue.
            for dc in range(NDC):
                sl = slice(dc * P, (dc + 1) * P)
                nc.sync.dma_start(wq_t[dc][:], wqt[sl, :])
                if dc < 2:
                    for q in range(4):
                        ps = slice(q * 32, (q + 1) * 32)
                        nc.sync.dma_start(
                            xt_t[dc][ps, :],
                            xt[dc * P + q * 32:dc * P + (q + 1) * 32, :])
                else:
                    nc.sync.dma_start(xt_t[dc][:], xt[sl, :])
            for dc in range(NDC):
                sl = slice(dc * P, (dc + 1) * P)
                nc.sync.dma_start(wk_t[dc][:], wkt[sl, :])
            for dc in range(NDC):
                sl = slice(dc * P, (dc + 1) * P)
                nc.sync.dma_start(wv_t[dc][:], wvt[sl, :])

            def qk_evac(dest, et, psA, psB, bias_sb):
                nc.vector.tensor_scalar_add(
                    dest[et][:, 0:1024], psA[:], bias_sb[:, et:et + 1])
                for i in range(2):
                    nc.vector.tensor_scalar_add(
                        dest[et][:, 1024 + i * 512:1024 + (i + 1) * 512],
                        psB[i][:], bias_sb[:, et:et + 1])

            def qk_psums(et):
                psA = pst.tile([P, 1024], F32, name=f"psA{et}", tag="sc", bufs=2)
                psB = [pst.tile([P, 512], F32, name=f"psB{et}_{i}", tag="acc",
                                bufs=4) for i in range(2)]
                outs = [psA[:, 0:512], psA[:, 512:1024], psB[0][:], psB[1][:]]
                return psA, psB, outs

            # Q: e-tile PAIRS concurrently — during the x-DMA ramp each
            # arriving chunk feeds 8 matmuls instead of 4, keeping PE busy
            for pair in range(2):
                ets = (2 * pair, 2 * pair + 1)
                ps = {et: qk_psums(et) for et in ets}
                for dc in range(NDC):
                    for et in ets:
                        lhsT = wq_t[dc][:, et * P:(et + 1) * P]
                        for sb_ in range(NSB):
                            nc.tensor.matmul(
                                ps[et][2][sb_], lhsT,
                                xt_t[dc][:, sb_ * 512:(sb_ + 1) * 512],
                                start=(dc == 0), stop=(dc == NDC - 1))
                for et in ets:
                    qk_evac(qt_t, et, ps[et][0], ps[et][1], bq_sb)

            # K: sequential e-tile groups (input already resident)
            for et in range(HPC):
                psA, psB, outs = qk_psums(et)
                for dc in range(NDC):
                    lhsT = wk_t[dc][:, et * P:(et + 1) * P]
                    for sb_ in range(NSB):
                        nc.tensor.matmul(
                            outs[sb_], lhsT,
                            xt_t[dc][:, sb_ * 512:(sb_ + 1) * 512],
                            start=(dc == 0), stop=(dc == NDC - 1))
                qk_evac(kt_t, et, psA, psB, bk_sb)

            # V: out[s_tile(128), e(512)] accumulated over d-chunks
            for st in range(NST):
                psv = pst.tile([P, 512], F32, name="psv", tag="acc", bufs=4)
                for dc in range(NDC):
                    nc.tensor.matmul(
                        psv[:], xt_t[dc][:, st * P:(st + 1) * P], wv_t[dc][:],
                        start=(dc == 0), stop=(dc == NDC - 1))
                nc.vector.tensor_copy(
                    v_t[st][:, :, 0:DK],
                    psv[:].rearrange("p (h w) -> p h w", h=HPC))

        # ------------------------------------------------------------------
        # Phase 2+3 fused, section-zippered.  Section s = (ib, hp):
        # emit score tiles for s interleaved with the AV chains of s-1 and
        # the transpose+output-projection of i-block (s-1)//2 when s-1 is a
        # hp==1 section.  The trailing drain runs av(3,1) t-major with the
        # final transposes/projections woven in.
        # ------------------------------------------------------------------
        with tc.tile_pool(name="p2", bufs=1) as p2, \
                tc.tile_pool(name="exps", bufs=36) as epool, \
                tc.tile_pool(name="small", bufs=8) as spool, \
                tc.tile_pool(name="yout", bufs=4) as ypool:
            o_t = [p2.tile([P, HPC * DK], BF, name=f"o{st}", tag=f"o{st}")
                   for st in range(NST)]
            wot_t = [p2.tile([P, D], BF, name=f"wot{ec}", tag=f"wot{ec}")
                     for ec in range(HPC)]
            ot_t = [p2.tile([P, S], BF, name=f"ot{ec}", tag=f"ot{ec}")
                    for ec in range(HPC)]
            for ec in range(HPC):
                nc.sync.dma_start(wot_t[ec][:], wot[ec * P:(ec + 1) * P, :])

            etiles = {}

            def emit_scores_tile(ib, hp, jt):
                pss = pst.tile([P, 1024], F32, name="pss", tag="sc", bufs=2)
                # band tiles only need i >= jt*128: slice N accordingly
                c0 = max(0, (jt - 4 * ib)) * P
                for k in range(2):
                    h = 2 * hp + k
                    nc.tensor.matmul(
                        pss[:, k * 512 + c0:(k + 1) * 512],
                        kt_t[h][:, jt * P:(jt + 1) * P],
                        qt_t[h][:, ib * 512 + c0:(ib + 1) * 512],
                        start=True, stop=True)
                et_t = epool.tile([P, 1024], BF, name="et", tag="exp")
                if jt < 4 * ib:
                    nc.scalar.activation(
                        et_t[:], pss[:], mybir.ActivationFunctionType.Exp)
                else:
                    s_off = jt - 4 * ib
                    # one call over both heads' unmasked regions
                    nc.scalar.activation(
                        et_t[:, s_off * P:1024], pss[:, s_off * P:1024],
                        mybir.ActivationFunctionType.Exp)
                    # zero the diag-masked part of both heads at once
                    et3 = et_t[:].rearrange("p (h w) -> p h w", h=2)
                    nc.vector.tensor_tensor(
                        et3[:, :, s_off * P:(s_off + 1) * P],
                        et3[:, :, s_off * P:(s_off + 1) * P],
                        tri[:, None, :].to_broadcast([P, 2, P]),
                        mybir.AluOpType.mult)
                etiles[(ib, hp)].append(et_t)

            def av_unit(ib, hp, k, t):
                h = 2 * hp + k
                it = 4 * ib + t
                po = pst.tile([P, 512], F32, name="po", tag="acc", bufs=4)
                ets = etiles[(ib, hp)]
                for jt in range(it + 1):
                    nc.tensor.matmul(
                        po[:, 0:DKP],
                        ets[jt][:, k * 512 + t * P:k * 512 + (t + 1) * P],
                        v_t[jt][:, h, :],
                        start=(jt == 0), stop=(jt == it))
                rec = spool.tile([P, 1], F32, name="rec", tag="rec")
                nc.vector.reciprocal(rec[:], po[:, DK:DKP])
                nc.vector.tensor_scalar_mul(
                    o_t[it][:, h * P:(h + 1) * P], po[:, 0:DK], rec[:])

            def tp_unit(st):
                for ec in range(HPC):
                    pt = pst.tile([P, P], BF, name="pt", tag="acc", bufs=4)
                    nc.tensor.transpose(
                        pt[:], o_t[st][:, ec * P:(ec + 1) * P], ident[:])
                    nc.vector.tensor_copy(
                        ot_t[ec][:, st * P:(st + 1) * P], pt[:])

            def oproj_unit(st, ob):
                py = pst.tile([P, 512], F32, name="py", tag="acc", bufs=4)
                for ec in range(HPC):
                    nc.tensor.matmul(
                        py[:], ot_t[ec][:, st * P:(st + 1) * P],
                        wot_t[ec][:, ob * 512:(ob + 1) * 512],
                        start=(ec == 0), stop=(ec == HPC - 1))
                y = ypool.tile([P, 512], BF, name="y", tag="y")
                nc.vector.tensor_copy(y[:], py[:])
                # two partition-halves on two queues: halves the latency of
                # the final s-tile's output flush
                nc.sync.dma_start(
                    outd[st * P:st * P + 64, ob * 512:(ob + 1) * 512],
                    y[0:64, :])
                nc.sync.dma_start(
                    outd[st * P + 64:(st + 1) * P, ob * 512:(ob + 1) * 512],
                    y[64:128, :])

            # alt-work units (cost ~ PE ns) for the zipper
            def alt_units(ib, hp):
                units = [(64 * (4 * ib + t + 1),
                          lambda k=k, t=t: av_unit(ib, hp, k, t))
                         for t in range(4) for k in range(2)]
                if hp == 1:
                    for t in range(4):
                        st = 4 * ib + t
                        units.append((400, lambda st=st: tp_unit(st)))
                    for t in range(4):
                        st = 4 * ib + t
                        for ob in range(NSB):
                            units.append(
                                (860, lambda st=st, ob=ob: oproj_unit(st, ob)))
                return units

            sections = [(ib, hp) for ib in range(NSB) for hp in range(2)]
            for s, (ib, hp) in enumerate(sections):
                etiles[(ib, hp)] = []
                units = alt_units(*sections[s - 1]) if s > 0 else []
                total = sum(c for c, _ in units)
                njt = 4 * ib + 4
                spent = 0
                ui = 0
                for jt in range(njt):
                    emit_scores_tile(ib, hp, jt)
                    target = total * (jt + 1) / njt
                    while ui < len(units) and spent < target:
                        c, fn = units[ui]
                        fn()
                        spent += c
                        ui += 1
                while ui < len(units):
                    ui_c, fn = units[ui]
                    fn()
                    ui += 1

            # drain: av(3,1) t-major with transposes/projections interleaved
            for t in range(4):
                for k in range(2):
                    av_unit(3, 1, k, t)
                if t >= 1:
                    tp_unit(12 + t - 1)
                    for ob in range(NSB):
                        oproj_unit(12 + t - 1, ob)
            tp_unit(15)
            for ob in range(NSB):
                oproj_unit(15, ob)

    nc.finalize()
    return nc


_NC_CACHE = {}


def _get_nc():
    if "nc" not in _NC_CACHE:
        _NC_CACHE["nc"] = _build_nc()
    return _NC_CACHE["nc"]


def _make_in_maps(x, wq, bq, wk, bk, wv, wo):
    scale = np.float32(1.0 / np.sqrt(DK))
    in_maps = []
    for c in range(NCORES):
        b = c // 4
        g = c % 4
        sl = slice(E * g, E * (g + 1))
        in_maps.append({
            "xt": np.ascontiguousarray(x[b].T).astype(BF16),
            "wqt": np.ascontiguousarray((wq[sl] * scale).T).astype(BF16),
            "wkt": np.ascontiguousarray(wk[sl].T).astype(BF16),
            "wvt": np.ascontiguousarray(wv[sl].T).astype(BF16),
            "wot": np.ascontiguousarray(wo[:, sl].T).astype(BF16),
            "bq": (bq[sl] * scale).astype(np.float32),
            "bk": bk[sl].astype(np.float32),
        })
    return in_maps


def _assemble(core_outs, wv_bias_vec):
    out = np.empty((B, S, D), np.float32)
    for b in range(B):
        acc = core_outs[4 * b].astype(np.float32)
        for g in range(1, 4):
            acc = acc + core_outs[4 * b + g].astype(np.float32)
        out[b] = acc + wv_bias_vec
    return out


def kernel(x, wq, bq, wk, bk, wv, bv, wo, bo, mask, _trace=False):
    x = np.asarray(x, dtype=np.float32)
    wq = np.asarray(wq, dtype=np.float32)
    bq = np.asarray(bq, dtype=np.float32)
    wk = np.asarray(wk, dtype=np.float32)
    bk = np.asarray(bk, dtype=np.float32)
    wv = np.asarray(wv, dtype=np.float32)
    bv = np.asarray(bv, dtype=np.float32)
    wo = np.asarray(wo, dtype=np.float32)
    bo = np.asarray(bo, dtype=np.float32)

    in_maps = _make_in_maps(x, wq, bq, wk, bk, wv, wo)
    nc = _get_nc()
    res = run_bass_kernel_spmd(nc, in_maps, core_ids=list(range(NCORES)),
                               trace=_trace)
    core_outs = [res.results[c]["out"] for c in range(NCORES)]
    # rows of softmax sum to 1 -> per-head V-bias contributes wo[:, sl] @ bv[sl]
    bias_vec = (bo + wo @ bv).astype(np.float32)
    out = _assemble(core_outs, bias_vec)
    if _trace:
        return out, res
    return out
